# revision 14
# baseline (speedup 1.0000x reference)
"""Trainium2 Bass kernel for nn_FFMCell: new_state = state*gamma + x (complex),
gamma = exp((-|a| + i*b) * j), plus int output j + i.

Sharding: T=4096 split across 8 cores (512 timesteps each). a/b tiny,
replicated (folded into gamma host-side).

Self-contained: relies only on the container's concourse install (on
PYTHONPATH), no files from the problem directory.
"""

import numpy as np

import concourse.bass as bass
import concourse.mybir as mybir
from concourse import bass_utils
from concourse.tile import TileContext

T, TRACE, CTX = 4096, 64, 64
NCORES = 8
TL = T // NCORES          # timesteps per core
F = TRACE * CTX           # flattened (trace, ctx) feature dim
P = 128                   # SBUF partitions
FTILE = 4096              # free-dim tile size
NT = TL // P              # t-tiles per core
NF = F // FTILE           # f-tiles per row block

_NC_CACHE = {}


def _legalize_single_wait(nc):
    """This toolchain's walrus accepts at most ONE sync-wait command per
    instruction; hoist extra waits onto standalone NoOps on the same engine
    queue (sequencers execute in order, so the NoOp gates the next inst)."""
    for fn in nc.m.functions:
        for bb in fn.blocks:
            out = []
            for inst in bb.instructions:
                si = inst.sync_info
                if si is not None and si.on_wait and len(si.on_wait) > 1:
                    waits = list(si.on_wait)
                    for k, w in enumerate(waits[:-1]):
                        out.append(mybir.InstNoOp(
                            name=f"{inst.name}-w{k}",
                            engine=inst.engine,
                            ins=[], outs=[],
                            sync_info=mybir.SyncInfo(on_wait=[w], on_update=[]),
                        ))
                    inst.sync_info = mybir.SyncInfo(
                        on_wait=[waits[-1]], on_update=list(si.on_update or []))
                out.append(inst)
            bb.instructions = out
    return nc


def _complex_madd(nc, pool, srt, sit, xrt, xit, grs, gis):
    """xrt,xit <- (srt,sit)*(grs,gis) + (xrt,xit) using 8 DVE tensor-tensor ops."""
    M = mybir.AluOpType
    V = nc.vector
    tmp = pool.tile([P, FTILE], mybir.dt.float32)
    tmp2 = pool.tile([P, FTILE], mybir.dt.float32)
    V.tensor_tensor(tmp[:], srt[:], grs, M.mult)
    V.tensor_tensor(tmp2[:], sit[:], gis, M.mult)
    V.tensor_tensor(xrt[:], xrt[:], tmp[:], M.add)
    V.tensor_tensor(xrt[:], xrt[:], tmp2[:], M.subtract)
    V.tensor_tensor(tmp[:], srt[:], gis, M.mult)
    V.tensor_tensor(tmp2[:], sit[:], grs, M.mult)
    V.tensor_tensor(xit[:], xit[:], tmp[:], M.add)
    V.tensor_tensor(xit[:], xit[:], tmp2[:], M.add)


def _build_fast():
    """gamma is a single [F] vector (all j equal): DMA'd once into one
    partition, broadcast across partitions on-chip (SBUF->SBUF), so HBM
    traffic is just the 4 payload tensors in + 2 out.

    Per tile: 6 DVE tensor-tensor ops + 2 GpSimd tensor-tensor ops
    (the two independent products for the imaginary part), balancing the
    engines under the DMA roofline."""
    nc = bass.Bass("TRN2", target_bir_lowering=False)
    f32 = mybir.dt.float32
    M = mybir.AluOpType
    sr = nc.dram_tensor("sr", [TL, F], f32, kind="ExternalInput")
    si = nc.dram_tensor("si", [TL, F], f32, kind="ExternalInput")
    xr = nc.dram_tensor("xr", [TL, F], f32, kind="ExternalInput")
    xi = nc.dram_tensor("xi", [TL, F], f32, kind="ExternalInput")
    gr = nc.dram_tensor("gr", [F], f32, kind="ExternalInput")
    gi = nc.dram_tensor("gi", [F], f32, kind="ExternalInput")
    o_r = nc.dram_tensor("o_r", [TL, F], f32, kind="ExternalOutput")
    o_i = nc.dram_tensor("o_i", [TL, F], f32, kind="ExternalOutput")

    with TileContext(nc) as tc:
        with (
            tc.tile_pool(name="g", bufs=1) as gpool,
            tc.tile_pool(name="io", bufs=2) as pool,
            tc.tile_pool(name="tmp", bufs=1) as tpool,
        ):
            grt = gpool.tile([P, F], f32)
            git = gpool.tile([P, F], f32)
            # gamma broadcast rides the SECOND HWDGE ring (ACT engine), in
            # halves, so it lands in parallel with the first input tiles on
            # the SP ring; first TTs only need gr[:, :H].
            H = F // 2
            nc.scalar.dma_start(grt[:, :H], gr[:H].partition_broadcast(P))
            nc.scalar.dma_start(git[:, :H], gi[:H].partition_broadcast(P))
            nc.scalar.dma_start(grt[:, H:], gr[H:].partition_broadcast(P))
            nc.scalar.dma_start(git[:, H:], gi[H:].partition_broadcast(P))
            V = nc.vector
            for it in range(NT):
                rsl = slice(it * P, (it + 1) * P)
                for jf in range(NF):
                    csl = slice(jf * FTILE, (jf + 1) * FTILE)
                    srt = pool.tile([P, FTILE], f32, name="srt")
                    sit = pool.tile([P, FTILE], f32, name="sit")
                    xrt = pool.tile([P, FTILE], f32, name="xrt")
                    xit = pool.tile([P, FTILE], f32, name="xit")
                    nc.sync.dma_start(srt[:], sr[rsl, csl])
                    nc.sync.dma_start(sit[:], si[rsl, csl])
                    nc.sync.dma_start(xrt[:], xr[rsl, csl])
                    nc.sync.dma_start(xit[:], xi[rsl, csl])
                    grs, gis = grt[:, csl], git[:, csl]
                    t1 = tpool.tile([P, FTILE], f32, name="t1")
                    t2 = tpool.tile([P, FTILE], f32, name="t2")
                    V.tensor_tensor(t1[:], srt[:], grs, M.mult)
                    V.tensor_tensor(t2[:], sit[:], gis, M.mult)
                    V.tensor_tensor(xrt[:], xrt[:], t1[:], M.add)
                    V.tensor_tensor(xrt[:], xrt[:], t2[:], M.subtract)
                    V.tensor_tensor(t1[:], srt[:], gis, M.mult)
                    V.tensor_tensor(t2[:], sit[:], grs, M.mult)
                    V.tensor_tensor(xit[:], xit[:], t1[:], M.add)
                    V.tensor_tensor(xit[:], xit[:], t2[:], M.add)
                    # split stores across both HWDGE rings
                    nc.sync.dma_start(o_r[rsl, csl], xrt[:])
                    nc.scalar.dma_start(o_i[rsl, csl], xit[:])
    return nc


def _build_generic():
    """Full per-timestep gamma tensors streamed from DRAM."""
    nc = bass.Bass("TRN2", target_bir_lowering=False)
    f32 = mybir.dt.float32
    sr = nc.dram_tensor("sr", [TL, F], f32, kind="ExternalInput")
    si = nc.dram_tensor("si", [TL, F], f32, kind="ExternalInput")
    xr = nc.dram_tensor("xr", [TL, F], f32, kind="ExternalInput")
    xi = nc.dram_tensor("xi", [TL, F], f32, kind="ExternalInput")
    gr = nc.dram_tensor("gr", [TL, F], f32, kind="ExternalInput")
    gi = nc.dram_tensor("gi", [TL, F], f32, kind="ExternalInput")
    o_r = nc.dram_tensor("o_r", [TL, F], f32, kind="ExternalOutput")
    o_i = nc.dram_tensor("o_i", [TL, F], f32, kind="ExternalOutput")

    with TileContext(nc) as tc:
        with tc.tile_pool(name="io", bufs=3) as pool:
            for it in range(NT):
                rsl = slice(it * P, (it + 1) * P)
                for jf in range(NF):
                    csl = slice(jf * FTILE, (jf + 1) * FTILE)
                    srt = pool.tile([P, FTILE], f32)
                    sit = pool.tile([P, FTILE], f32)
                    xrt = pool.tile([P, FTILE], f32)
                    xit = pool.tile([P, FTILE], f32)
                    grt = pool.tile([P, FTILE], f32)
                    git = pool.tile([P, FTILE], f32)
                    nc.sync.dma_start(srt[:], sr[rsl, csl])
                    nc.sync.dma_start(sit[:], si[rsl, csl])
                    nc.sync.dma_start(xrt[:], xr[rsl, csl])
                    nc.sync.dma_start(xit[:], xi[rsl, csl])
                    nc.sync.dma_start(grt[:], gr[rsl, csl])
                    nc.sync.dma_start(git[:], gi[rsl, csl])
                    _complex_madd(nc, pool, srt, sit, xrt, xit,
                                  grt[:], git[:])
                    nc.sync.dma_start(o_r[rsl, csl], xrt[:])
                    nc.sync.dma_start(o_i[rsl, csl], xit[:])
    return nc


def _get_nc(variant):
    if variant not in _NC_CACHE:
        nc = _build_fast() if variant == "fast" else _build_generic()
        _NC_CACHE[variant] = _legalize_single_wait(nc)
    return _NC_CACHE[variant]


def _run(nc, in_maps, **kwargs):
    return bass_utils.run_bass_kernel_spmd(
        nc, in_maps, core_ids=list(range(NCORES)), **kwargs
    )


def kernel(a, b, state_re, state_im, x_re, x_im, i, j, _run_kwargs=None):
    a = np.asarray(a, np.float32)
    b = np.asarray(b, np.float32)
    state_re = np.ascontiguousarray(state_re, np.float32)
    state_im = np.ascontiguousarray(state_im, np.float32)
    x_re = np.ascontiguousarray(x_re, np.float32)
    x_im = np.ascontiguousarray(x_im, np.float32)
    i = np.asarray(i, np.int32)
    j = np.asarray(j, np.int32)

    sr = state_re.reshape(NCORES, TL, F)
    si = state_im.reshape(NCORES, TL, F)
    xr = x_re.reshape(NCORES, TL, F)
    xi = x_im.reshape(NCORES, TL, F)

    jf = j.astype(np.float32)
    na = -np.abs(a)[:, None]                      # [TRACE, 1]

    if np.all(j == j[0]):
        t0 = np.float32(jf[0])
        e = np.exp(na * t0, dtype=np.float32)     # [TRACE, 1]
        c = np.cos(b * t0, dtype=np.float32)      # [CTX]
        s = np.sin(b * t0, dtype=np.float32)
        g_re = (e * c[None, :]).astype(np.float32).reshape(F)
        g_im = (e * s[None, :]).astype(np.float32).reshape(F)
        nc = _get_nc("fast")
        in_maps = [
            {"sr": sr[cid], "si": si[cid], "xr": xr[cid], "xi": xi[cid],
             "gr": g_re, "gi": g_im}
            for cid in range(NCORES)
        ]
    else:
        tcol = jf[:, None]                        # [T, 1]
        e = np.exp(na[None, :, :] * tcol[:, :, None], dtype=np.float32)  # [T,TRACE,1]
        c = np.cos(b[None, :] * tcol, dtype=np.float32)                  # [T,CTX]
        s = np.sin(b[None, :] * tcol, dtype=np.float32)
        g_re = (e * c[:, None, :]).astype(np.float32).reshape(NCORES, TL, F)
        g_im = (e * s[:, None, :]).astype(np.float32).reshape(NCORES, TL, F)
        nc = _get_nc("generic")
        in_maps = [
            {"sr": sr[cid], "si": si[cid], "xr": xr[cid], "xi": xi[cid],
             "gr": np.ascontiguousarray(g_re[cid]),
             "gi": np.ascontiguousarray(g_im[cid])}
            for cid in range(NCORES)
        ]

    res = _run(nc, in_maps, **(_run_kwargs or {}))
    out_re = np.concatenate(
        [r["o_r"].reshape(TL, TRACE, CTX) for r in res.results], axis=0
    )
    out_im = np.concatenate(
        [r["o_i"].reshape(TL, TRACE, CTX) for r in res.results], axis=0
    )
    if _run_kwargs is not None:
        kernel.last_results = res
    return out_re, out_im, (j + i).astype(np.int32)


# revision 15
# speedup vs baseline: 1.2065x; 1.2065x over previous
"""Trainium2 Bass kernel for nn_FFMCell: new_state = state*gamma + x (complex),
gamma = exp((-|a| + i*b) * j), plus int output j + i.

Sharding: T=4096 split across 8 cores (512 timesteps each). a/b tiny,
replicated (folded into gamma host-side).

Self-contained: relies only on the container's concourse install (on
PYTHONPATH), no files from the problem directory.
"""

import numpy as np

import concourse.bass as bass
import concourse.mybir as mybir
from concourse import bass_utils
from concourse.tile import TileContext

T, TRACE, CTX = 4096, 64, 64
NCORES = 8
TL = T // NCORES          # timesteps per core
F = TRACE * CTX           # flattened (trace, ctx) feature dim
P = 128                   # SBUF partitions
FTILE = 2048              # free-dim tile size
NT = TL // P              # t-tiles per core
NF = F // FTILE           # f-tiles per row block

_NC_CACHE = {}


def _legalize_single_wait(nc):
    """This toolchain's walrus accepts at most ONE sync-wait command per
    instruction; hoist extra waits onto standalone NoOps on the same engine
    queue (sequencers execute in order, so the NoOp gates the next inst)."""
    for fn in nc.m.functions:
        for bb in fn.blocks:
            out = []
            for inst in bb.instructions:
                si = inst.sync_info
                if si is not None and si.on_wait and len(si.on_wait) > 1:
                    waits = list(si.on_wait)
                    for k, w in enumerate(waits[:-1]):
                        out.append(mybir.InstNoOp(
                            name=f"{inst.name}-w{k}",
                            engine=inst.engine,
                            ins=[], outs=[],
                            sync_info=mybir.SyncInfo(on_wait=[w], on_update=[]),
                        ))
                    inst.sync_info = mybir.SyncInfo(
                        on_wait=[waits[-1]], on_update=list(si.on_update or []))
                out.append(inst)
            bb.instructions = out
    return nc


def _complex_madd(nc, pool, srt, sit, xrt, xit, grs, gis):
    """xrt,xit <- (srt,sit)*(grs,gis) + (xrt,xit) using 8 DVE tensor-tensor ops."""
    M = mybir.AluOpType
    V = nc.vector
    tmp = pool.tile([P, FTILE], mybir.dt.float32)
    tmp2 = pool.tile([P, FTILE], mybir.dt.float32)
    V.tensor_tensor(tmp[:], srt[:], grs, M.mult)
    V.tensor_tensor(tmp2[:], sit[:], gis, M.mult)
    V.tensor_tensor(xrt[:], xrt[:], tmp[:], M.add)
    V.tensor_tensor(xrt[:], xrt[:], tmp2[:], M.subtract)
    V.tensor_tensor(tmp[:], srt[:], gis, M.mult)
    V.tensor_tensor(tmp2[:], sit[:], grs, M.mult)
    V.tensor_tensor(xit[:], xit[:], tmp[:], M.add)
    V.tensor_tensor(xit[:], xit[:], tmp2[:], M.add)


def _build_fast():
    """gamma is a single [F] vector (all j equal): DMA'd once into one
    partition, broadcast across partitions on-chip (SBUF->SBUF), so HBM
    traffic is just the 4 payload tensors in + 2 out.

    Per tile: 6 DVE tensor-tensor ops + 2 GpSimd tensor-tensor ops
    (the two independent products for the imaginary part), balancing the
    engines under the DMA roofline."""
    nc = bass.Bass("TRN2", target_bir_lowering=False)
    f32 = mybir.dt.float32
    M = mybir.AluOpType
    sr = nc.dram_tensor("sr", [TL, F], f32, kind="ExternalInput")
    si = nc.dram_tensor("si", [TL, F], f32, kind="ExternalInput")
    xr = nc.dram_tensor("xr", [TL, F], f32, kind="ExternalInput")
    xi = nc.dram_tensor("xi", [TL, F], f32, kind="ExternalInput")
    gr = nc.dram_tensor("gr", [F], f32, kind="ExternalInput")
    gi = nc.dram_tensor("gi", [F], f32, kind="ExternalInput")
    o_r = nc.dram_tensor("o_r", [TL, F], f32, kind="ExternalOutput")
    o_i = nc.dram_tensor("o_i", [TL, F], f32, kind="ExternalOutput")

    with TileContext(nc) as tc:
        with (
            tc.tile_pool(name="g", bufs=1) as gpool,
            tc.tile_pool(name="io", bufs=4) as pool,
            tc.tile_pool(name="tmp", bufs=1) as tpool,
        ):
            grt = gpool.tile([P, F], f32)
            git = gpool.tile([P, F], f32)
            # gamma broadcast rides the SECOND HWDGE ring (ACT engine), in
            # halves, so it lands in parallel with the first input tiles on
            # the SP ring; first TTs only need gr[:, :H].
            H = F // 2
            nc.scalar.dma_start(grt[:, :H], gr[:H].partition_broadcast(P))
            nc.scalar.dma_start(git[:, :H], gi[:H].partition_broadcast(P))
            nc.scalar.dma_start(grt[:, H:], gr[H:].partition_broadcast(P))
            nc.scalar.dma_start(git[:, H:], gi[H:].partition_broadcast(P))
            V = nc.vector
            for it in range(NT):
                rsl = slice(it * P, (it + 1) * P)
                for jf in range(NF):
                    csl = slice(jf * FTILE, (jf + 1) * FTILE)
                    srt = pool.tile([P, FTILE], f32, name="srt")
                    sit = pool.tile([P, FTILE], f32, name="sit")
                    xrt = pool.tile([P, FTILE], f32, name="xrt")
                    xit = pool.tile([P, FTILE], f32, name="xit")
                    nc.sync.dma_start(srt[:], sr[rsl, csl])
                    nc.sync.dma_start(sit[:], si[rsl, csl])
                    nc.sync.dma_start(xrt[:], xr[rsl, csl])
                    nc.sync.dma_start(xit[:], xi[rsl, csl])
                    grs, gis = grt[:, csl], git[:, csl]
                    t1 = tpool.tile([P, FTILE], f32, name="t1")
                    t2 = tpool.tile([P, FTILE], f32, name="t2")
                    V.tensor_tensor(t1[:], srt[:], grs, M.mult)
                    V.tensor_tensor(t2[:], sit[:], gis, M.mult)
                    V.tensor_tensor(xrt[:], xrt[:], t1[:], M.add)
                    V.tensor_tensor(xrt[:], xrt[:], t2[:], M.subtract)
                    V.tensor_tensor(t1[:], srt[:], gis, M.mult)
                    V.tensor_tensor(t2[:], sit[:], grs, M.mult)
                    V.tensor_tensor(xit[:], xit[:], t1[:], M.add)
                    V.tensor_tensor(xit[:], xit[:], t2[:], M.add)
                    # split stores across both HWDGE rings
                    nc.sync.dma_start(o_r[rsl, csl], xrt[:])
                    nc.scalar.dma_start(o_i[rsl, csl], xit[:])
    return nc


def _build_generic():
    """Full per-timestep gamma tensors streamed from DRAM."""
    nc = bass.Bass("TRN2", target_bir_lowering=False)
    f32 = mybir.dt.float32
    sr = nc.dram_tensor("sr", [TL, F], f32, kind="ExternalInput")
    si = nc.dram_tensor("si", [TL, F], f32, kind="ExternalInput")
    xr = nc.dram_tensor("xr", [TL, F], f32, kind="ExternalInput")
    xi = nc.dram_tensor("xi", [TL, F], f32, kind="ExternalInput")
    gr = nc.dram_tensor("gr", [TL, F], f32, kind="ExternalInput")
    gi = nc.dram_tensor("gi", [TL, F], f32, kind="ExternalInput")
    o_r = nc.dram_tensor("o_r", [TL, F], f32, kind="ExternalOutput")
    o_i = nc.dram_tensor("o_i", [TL, F], f32, kind="ExternalOutput")

    with TileContext(nc) as tc:
        with tc.tile_pool(name="io", bufs=3) as pool:
            for it in range(NT):
                rsl = slice(it * P, (it + 1) * P)
                for jf in range(NF):
                    csl = slice(jf * FTILE, (jf + 1) * FTILE)
                    srt = pool.tile([P, FTILE], f32)
                    sit = pool.tile([P, FTILE], f32)
                    xrt = pool.tile([P, FTILE], f32)
                    xit = pool.tile([P, FTILE], f32)
                    grt = pool.tile([P, FTILE], f32)
                    git = pool.tile([P, FTILE], f32)
                    nc.sync.dma_start(srt[:], sr[rsl, csl])
                    nc.sync.dma_start(sit[:], si[rsl, csl])
                    nc.sync.dma_start(xrt[:], xr[rsl, csl])
                    nc.sync.dma_start(xit[:], xi[rsl, csl])
                    nc.sync.dma_start(grt[:], gr[rsl, csl])
                    nc.sync.dma_start(git[:], gi[rsl, csl])
                    _complex_madd(nc, pool, srt, sit, xrt, xit,
                                  grt[:], git[:])
                    nc.sync.dma_start(o_r[rsl, csl], xrt[:])
                    nc.sync.dma_start(o_i[rsl, csl], xit[:])
    return nc


def _get_nc(variant):
    if variant not in _NC_CACHE:
        nc = _build_fast() if variant == "fast" else _build_generic()
        _NC_CACHE[variant] = _legalize_single_wait(nc)
    return _NC_CACHE[variant]


def _run(nc, in_maps, **kwargs):
    return bass_utils.run_bass_kernel_spmd(
        nc, in_maps, core_ids=list(range(NCORES)), **kwargs
    )


def kernel(a, b, state_re, state_im, x_re, x_im, i, j, _run_kwargs=None):
    a = np.asarray(a, np.float32)
    b = np.asarray(b, np.float32)
    state_re = np.ascontiguousarray(state_re, np.float32)
    state_im = np.ascontiguousarray(state_im, np.float32)
    x_re = np.ascontiguousarray(x_re, np.float32)
    x_im = np.ascontiguousarray(x_im, np.float32)
    i = np.asarray(i, np.int32)
    j = np.asarray(j, np.int32)

    sr = state_re.reshape(NCORES, TL, F)
    si = state_im.reshape(NCORES, TL, F)
    xr = x_re.reshape(NCORES, TL, F)
    xi = x_im.reshape(NCORES, TL, F)

    jf = j.astype(np.float32)
    na = -np.abs(a)[:, None]                      # [TRACE, 1]

    if np.all(j == j[0]):
        t0 = np.float32(jf[0])
        e = np.exp(na * t0, dtype=np.float32)     # [TRACE, 1]
        c = np.cos(b * t0, dtype=np.float32)      # [CTX]
        s = np.sin(b * t0, dtype=np.float32)
        g_re = (e * c[None, :]).astype(np.float32).reshape(F)
        g_im = (e * s[None, :]).astype(np.float32).reshape(F)
        nc = _get_nc("fast")
        in_maps = [
            {"sr": sr[cid], "si": si[cid], "xr": xr[cid], "xi": xi[cid],
             "gr": g_re, "gi": g_im}
            for cid in range(NCORES)
        ]
    else:
        tcol = jf[:, None]                        # [T, 1]
        e = np.exp(na[None, :, :] * tcol[:, :, None], dtype=np.float32)  # [T,TRACE,1]
        c = np.cos(b[None, :] * tcol, dtype=np.float32)                  # [T,CTX]
        s = np.sin(b[None, :] * tcol, dtype=np.float32)
        g_re = (e * c[:, None, :]).astype(np.float32).reshape(NCORES, TL, F)
        g_im = (e * s[:, None, :]).astype(np.float32).reshape(NCORES, TL, F)
        nc = _get_nc("generic")
        in_maps = [
            {"sr": sr[cid], "si": si[cid], "xr": xr[cid], "xi": xi[cid],
             "gr": np.ascontiguousarray(g_re[cid]),
             "gi": np.ascontiguousarray(g_im[cid])}
            for cid in range(NCORES)
        ]

    res = _run(nc, in_maps, **(_run_kwargs or {}))
    out_re = np.concatenate(
        [r["o_r"].reshape(TL, TRACE, CTX) for r in res.results], axis=0
    )
    out_im = np.concatenate(
        [r["o_i"].reshape(TL, TRACE, CTX) for r in res.results], axis=0
    )
    if _run_kwargs is not None:
        kernel.last_results = res
    return out_re, out_im, (j + i).astype(np.int32)
